# revision 21
# baseline (speedup 1.0000x reference)
"""Trainium2 Bass kernel for nn_LowFreqPenaltyLoss.

Computes mean(|einsum('ih,nchw,jw->ncij', Ch, delta, Cw)|) for
delta [256, 3, 256, 256] f32, Ch/Cw the 8x256 unnormalized DCT-II bases.

Strategy (data-parallel over batch, 8 cores):
  - each core gets 32 batches = 96 images [256, 256] (24 MiB), streamed in
    16-image groups via 2 MiB SWDGE DMAs that cast f32->bf16 inline (the
    problem is memory-bound; measured ~390 GB/s/core, SDMA engines 97-99%
    busy during the stream).
  - stage A (contract h): psum[32q+i, (e,w)] += ChT_pad[h',i].T @ img per
    image pair, 4 pairs packed per PSUM bank at partition offsets
    {0,32,64,96} via col tile_position. Weights are Ch padded with zero
    columns to M=32 so the full bank is written (garbage-free).
  - copy bank -> SBUF (ACT, casts to bf16), PE-transpose 128x128 chunks
    (each into its own PSUM bank: transpose-mode output must start at a
    bank boundary on HW), DVE copies out, stage B (contract w):
    out2[(q,i), j] += T.T @ CwT, then fused |.|+sum on DVE into a
    per-partition accumulator.
  - final: ones-matmul partition reduction scaled by 1/49152; host sums
    the 8 per-core partials. bf16 inputs + f32 PSUM accumulation give
    ~2e-4 relative error on the final scalar.
"""

import sys
import types

for _p in ("/root/.axon_site/_ro/trn_rl_repo", "/opt/trn_rl_repo"):
    if _p not in sys.path:
        sys.path.append(_p)

import numpy as np
from contextlib import ExitStack

import concourse.bass as bass
import concourse.tile as tile
from concourse import mybir, bass_utils
from concourse._compat import with_exitstack
from concourse.vector_clock import ScopedClock

# ---------------------------------------------------------------------------
# Workarounds for this image.
# ---------------------------------------------------------------------------

# walrus on this image rejects >1 sync-wait on one CTRL instruction; split the
# Tile exit-drain's waits across follow-up nops (same engine, program order).
# Also: the stock tail (barrier + per-sem clear + barrier) costs ~8-10us of
# EVSEM butterfly at kernel end. The kernel is one-shot per NEFF execution and
# NRT re-initialises semaphores per execution, so keep only the drain + DMA
# completion waits.
_ORIG_DAB = tile.TileContext._drain_and_barrier
_USE_STOCK_TAIL = False


def _patched_drain_and_barrier(self, tick_clock, wait_clock):
    if _USE_STOCK_TAIL:
        return _ORIG_DAB(self, tick_clock, wait_clock)
    nc = self.nc
    drain_inst = nc.sync.drain()
    wait_clock.add_sem_waits(
        drain_inst.ins, ScopedClock({None: tick_clock.global_clock})
    )
    si = drain_inst.ins.sync_info
    waits = list(si.on_wait) if si and si.on_wait else []
    if len(waits) > 1:
        drain_inst.ins.sync_info = mybir.SyncInfo(
            on_wait=[waits[0]], on_update=list(si.on_update or [])
        )
        # Spread the remaining waits round-robin across engines: each NOP
        # costs ~60ns of issue latency and a serial ladder on one engine
        # adds ~1us to the tail; the NEFF fini barrier joins all engines
        # anyway, so any engine can carry any wait.
        wait_engines = [nc.sync, nc.vector, nc.scalar, nc.tensor, nc.gpsimd]
        for k, w in enumerate(waits[1:]):
            eng = wait_engines[k % len(wait_engines)]
            nop = eng.nop(nofuse=True, hint="drain_wait_split")
            nop.ins.sync_info = mybir.SyncInfo(on_wait=[w], on_update=[])
    popped = nc._tile_sem_poison_stack.pop()
    assert popped is self._sem_poison


tile.TileContext._drain_and_barrier = _patched_drain_and_barrier

# zero-egress container: profiling artifact upload must stay local.
bass_utils.upload_artifacts = lambda d: d


def _strip_main_barrier(nc):
    """Drop the prologue/epilogue all-engine barriers in 'main' plus the dead
    const-ap memsets. The barriers' only role is to fence the framework
    preamble (per-engine table loads) from the kernel, but per-engine program
    order already covers the table loads; nothing reads the const tiles
    (verified: only the 4 Memsets reference 'const-*' in the BIR). The first
    main-block instruction defines the profiler's first_useful_time, so the
    memsets also pad the measured window. Saves ~3-4us total."""
    for fn in nc.m.functions:
        for bb in fn.blocks:
            if bb.name != "main":
                continue
            bb.instructions[:] = [
                i for i in bb.instructions
                if not isinstance(
                    i,
                    (mybir.InstEventSemaphore, mybir.InstDrain, mybir.InstMemset),
                )
            ]


def _split_multi_waits(nc):
    """walrus on this image rejects >1 sync-wait per instruction: hoist extra
    waits onto fresh NoOps inserted just before, on the same engine."""
    for fn in nc.m.functions:
        for bb in fn.blocks:
            new_insts = []
            for inst in bb.instructions:
                si = inst.sync_info
                waits = list(si.on_wait) if si and si.on_wait else []
                if len(waits) > 1:
                    for w in waits[:-1]:
                        nop = mybir.InstNoOp(
                            name=nc.get_next_instruction_name(),
                            sync_info=mybir.SyncInfo(on_wait=[w], on_update=[]),
                            bass_nofuse=True,
                            engine=inst.engine,
                        )
                        new_insts.append(nop)
                    inst.sync_info = mybir.SyncInfo(
                        on_wait=[waits[-1]], on_update=list(si.on_update or [])
                    )
                new_insts.append(inst)
            bb.instructions[:] = new_insts

# ---------------------------------------------------------------------------
# Problem constants (hardcoded; kernel.py must be self-contained).
# ---------------------------------------------------------------------------

B, C, H, W = 256, 3, 256, 256
LOW_A = LOW_B = 8
N_CORES = 8
IMGS_PER_CORE = (B // N_CORES) * C          # 96
N_GROUPS = IMGS_PER_CORE // 8               # 12 groups of 8 images (4 pairs)
TOTAL_LOW = B * C * LOW_A * LOW_B           # 49152 -> mean divisor

F32 = mybir.dt.float32
BF16 = mybir.dt.bfloat16


def _dct_basis(K, N):
    n = np.arange(N, dtype=np.float64)
    k = np.arange(K, dtype=np.float64)
    return (2.0 * np.cos(np.pi * (2.0 * n[None, :] + 1.0) * k[:, None] / (2.0 * N))).astype(
        np.float32
    )


def _make_consts():
    Ch = _dct_basis(LOW_A, H)   # [8, 256]
    Cw = _dct_basis(LOW_B, W)   # [8, 256]
    # Rows are loaded 2-per-partition (partition p holds image rows 2p and
    # 2p+1 -> 2 KiB contiguous HBM reads per descriptor instead of 1 KiB).
    # chtp[hh, p, i] = Ch[i, 2p+hh], padded to 32 cols with zeros; the h
    # contraction becomes two accumulating passes hh=0,1 over 128 partitions.
    # E15-free row map: partitions 92-95 and 124-127 are served exclusively
    # by SDMA engine 15, which is an intermittent (+10-13%/packet) straggler
    # that gates the whole tail in ~half of runs, on both DGE paths. So no
    # input bytes land there: partitions 0..91 hold row pairs (2p, 2p+1)
    # of rows 0..183, partitions 96..123 hold pairs of rows 184..239, and
    # rows 240..255 go 1-per-partition to partitions 0..15 as a third,
    # 16-row accumulation pass. chtpE[hh, p, i] = Ch[i, row(p, hh)].
    chtp = np.zeros((2, 128, 32), np.float32)
    for hh in range(2):
        chtp[hh, 0:92, :8] = Ch[:, hh:184:2].T
        chtp[hh, 96:124, :8] = Ch[:, 184 + hh:240:2].T
    chtp3 = np.zeros((16, 32), np.float32)
    chtp3[:, :8] = Ch[:, 240:256].T
    # cwt[wc, p, j] = Cw[j, wc*128+p]
    cwt = np.zeros((2, 128, 8), np.float32)
    for wc in range(2):
        cwt[wc] = Cw[:, wc * 128:(wc + 1) * 128].T
    import ml_dtypes
    bf16 = ml_dtypes.bfloat16
    ident = np.eye(128, dtype=bf16)
    sumw = np.full((128, 1), 1.0 / TOTAL_LOW, np.float32)
    zeros = np.zeros((4, 4096), bf16)
    return chtp.astype(bf16), chtp3.astype(bf16), cwt.astype(bf16), ident, sumw, zeros


CHTP, CHTP3, CWT, IDENT, SUMW, ZEROS = _make_consts()


# ---------------------------------------------------------------------------
# Kernel body (per core; SPMD over 8 cores).
# ---------------------------------------------------------------------------

@with_exitstack
def _lowfreq_kernel(ctx: ExitStack, tc, out_ap, delta_ap, chtp_ap, chtp3_ap,
                    cwt_ap, ident_ap, sumw_ap, zeros_ap):
    nc = tc.nc

    const_pool = ctx.enter_context(tc.tile_pool(name="const", bufs=1))
    in_pool = ctx.enter_context(tc.tile_pool(name="input", bufs=8))
    rem_pool = ctx.enter_context(tc.tile_pool(name="rem", bufs=8))
    sS_pool = ctx.enter_context(tc.tile_pool(name="sS", bufs=3))
    tS_pool = ctx.enter_context(tc.tile_pool(name="tS", bufs=3))
    red_pool = ctx.enter_context(tc.tile_pool(name="red", bufs=2))
    acc_pool = ctx.enter_context(tc.tile_pool(name="acc", bufs=1))
    psA_pool = ctx.enter_context(tc.tile_pool(name="psA", bufs=3, space="PSUM"))
    psT_pool = ctx.enter_context(tc.tile_pool(name="psT", bufs=3, space="PSUM"))
    ps2_pool = ctx.enter_context(tc.tile_pool(name="ps2", bufs=2, space="PSUM"))

    # constants — issued on the ACT HWDGE ring (nc.scalar) so the Sync FIFO
    # carries only the input stream and the first input DMA issues ~2.6us
    # earlier (HWDGE DMAs are FIFO per issuing engine).
    chtp = const_pool.tile([128, 2, 32], BF16)      # [p, hh, i]
    nc.scalar.dma_start(chtp[:], chtp_ap.rearrange("hh p i -> p hh i"))
    chtp3 = const_pool.tile([16, 32], BF16)         # [p, i] rows 240+p
    nc.scalar.dma_start(chtp3[:], chtp3_ap)
    cwt = const_pool.tile([128, 2, 8], BF16)        # [p, wc, j]
    nc.scalar.dma_start(cwt[:], cwt_ap.rearrange("wc p j -> p wc j"))
    ident = const_pool.tile([128, 128], BF16)
    nc.scalar.dma_start(ident[:], ident_ap)
    sumw = const_pool.tile([128, 1], F32)
    nc.scalar.dma_start(sumw[:], sumw_ap)

    acc = acc_pool.tile([128, 1], F32)
    nc.vector.memset(acc[:], 0.0)

    def do_half(gsel, grem, moff):
        # stage A: contract h in three accumulating passes: hh=0,1 over the
        # 120 written partitions (rows 0..239, zero weights on the 8
        # engine-15 partitions, which are memset to 0), then a 16-row pass
        # for rows 240..255 from the remainder tile. hh-major order so the
        # four col-groups' matmuls can stream concurrently through the PE.
        # bank[32q+i, (e,w)].
        bankA = psA_pool.tile([128, 512], F32)
        for hh in range(2):
            for qq in range(4):
                nc.tensor.matmul(
                    bankA[32 * qq:32 * qq + 32, :],
                    lhsT=chtp[0:124, hh, :],
                    rhs=gsel[0:124, moff + qq, :, hh, :],
                    start=(hh == 0),
                    stop=False,
                    tile_position=(0, 32 * qq),
                    # CoreSim's zero-region tracker is bank-granular and
                    # flags the four concurrent per-partition col-groups;
                    # HW has_written state is per-element (verified on HW).
                    skip_group_check=True,
                )
        for qq in range(4):
            nc.tensor.matmul(
                bankA[32 * qq:32 * qq + 32, :],
                lhsT=chtp3[:],
                rhs=grem[:, moff + qq, :, :],
                start=False,
                stop=True,
                tile_position=(0, 32 * qq),
                skip_group_check=True,
            )

        # PSUM -> SBUF with f32->bf16 cast (ACT engine)
        sS = sS_pool.tile([128, 512], BF16)
        nc.scalar.copy(sS[:], bankA[:])

        # stage B: 4 PE transposes (own PSUM tiles: transpose-mode output
        # must start at a bank boundary on HW), DVE copies out, then
        # contract w into ps2 (e0 -> cols 0:8, e1 -> cols 8:16)
        tps = []
        for c in range(4):
            tp = psT_pool.tile([128, 128], BF16, tag="tp")
            nc.tensor.transpose(
                tp[:],
                sS[:, 128 * c:128 * c + 128],
                ident[:],
            )
            tps.append(tp)
        tSb = tS_pool.tile([128, 512], BF16)
        for c in range(4):
            nc.vector.tensor_copy(tSb[:, 128 * c:128 * c + 128], tps[c][:])

        ps2 = ps2_pool.tile([128, 16], F32)
        for e in range(2):
            for wc in range(2):
                c = 2 * e + wc
                nc.tensor.matmul(
                    ps2[:, 8 * e:8 * e + 8],
                    lhsT=tSb[:, 128 * c:128 * c + 128],
                    rhs=cwt[:, wc, :],
                    start=(wc == 0),
                    stop=(wc == 1),
                )
        red = red_pool.tile([128, 1], F32)
        nc.vector.tensor_reduce(
            red[:], ps2[:], axis=mybir.AxisListType.X,
            op=mybir.AluOpType.add, apply_absolute_value=True,
        )
        nc.vector.tensor_add(acc[:], acc[:], red[:])

    # Loads: twelve 8-image units [p, m(pair), e, hh, w] f32 via HWDGE
    # (nc.sync). Partition p holds image rows 2p and 2p+1 so each DMA
    # descriptor reads 2 KiB contiguous from HBM. HWDGE rather than
    # SWDGE-with-cast: the SWDGE descriptor rings live in SBUF on partition
    # lines whose AXI port also serves SDMA engine 15, which intermittently
    # makes engine 15 a +10-12us straggler that gates the whole tail
    # (observed ~50% of runs). HWDGE has no SBUF descriptor ring. Stage A
    # eats the f32 directly (PE fp32 LOW/HIGH passes, ~2x bf16 time --
    # still far below the stream time); 8-image units keep PE work arriving
    # every ~2.7us so the HAM clock gate stays open near the stream end,
    # and the post-stream critical path is a single 8-image unit.
    for u in range(12):
        # 8-image unit, SWDGE loads with inline f32->bf16 cast, 2 KiB
        # contiguous HBM reads per descriptor, nothing on engine 15's
        # partitions. The memsets zero those partitions so the zero-weight
        # stage-A columns multiply 0, not NaN garbage.
        gb = in_pool.tile([128, 4, 2, 2, 256], BF16)
        # partitions 92-95 are inside the [0:124) contraction range but get
        # no input bytes (engine-15 partitions): zero them via a tiny 32 KiB
        # DMA on the ACT HWDGE ring (zero x zero-weight, engine 15 only ever
        # moves these). Partitions 124-127 are excluded by the [0:124) range.
        nc.scalar.dma_start(gb[92:96, :, :, :, :], zeros_ap)
        src = delta_ap[8 * u:8 * u + 8, :, :]
        nc.gpsimd.dma_start(
            gb[0:92, :, :, :, :],
            src[:, 0:184, :].rearrange("(m e) (p hh) w -> p (m e) (hh w)",
                                       m=4, e=2, p=92, hh=2),
        )
        nc.gpsimd.dma_start(
            gb[96:124, :, :, :, :],
            src[:, 184:240, :].rearrange("(m e) (p hh) w -> p (m e) (hh w)",
                                         m=4, e=2, p=28, hh=2),
        )
        grem = rem_pool.tile([16, 4, 2, 256], BF16)
        nc.gpsimd.dma_start(
            grem[:],
            src[:, 240:256, :].rearrange("(m e) p w -> p (m e) w",
                                         m=4, e=2, p=16),
        )
        do_half(gb, grem, 0)

    # final partition reduction: out = acc.T @ sumw = sum_p acc[p] / 49152
    fout = ps2_pool.tile([1, 1], F32, tag="ps2")
    nc.tensor.matmul(fout[:], lhsT=acc[:], rhs=sumw[:], start=True, stop=True)
    fsb = red_pool.tile([1, 1], F32)
    nc.vector.tensor_copy(fsb[:], fout[:])
    nc.sync.dma_start(out_ap, fsb[:])


# ---------------------------------------------------------------------------
# Build + run.
# ---------------------------------------------------------------------------

_CACHED_NC = None


def _build(for_sim=False):
    global _CACHED_NC, _USE_STOCK_TAIL
    if not for_sim and _CACHED_NC is not None:
        return _CACHED_NC
    _USE_STOCK_TAIL = for_sim
    nc = bass.Bass("TRN2", target_bir_lowering=False, debug=False)
    delta = nc.dram_tensor("delta", [IMGS_PER_CORE, H, W], F32, kind="ExternalInput")
    chtp = nc.dram_tensor("chtp", list(CHTP.shape), BF16, kind="ExternalInput")
    chtp3 = nc.dram_tensor("chtp3", list(CHTP3.shape), BF16, kind="ExternalInput")
    cwt = nc.dram_tensor("cwt", list(CWT.shape), BF16, kind="ExternalInput")
    ident = nc.dram_tensor("ident", list(IDENT.shape), BF16, kind="ExternalInput")
    sumw = nc.dram_tensor("sumw", list(SUMW.shape), F32, kind="ExternalInput")
    zeros = nc.dram_tensor("zeros", list(ZEROS.shape), BF16, kind="ExternalInput")
    out = nc.dram_tensor("out", [1, 1], F32, kind="ExternalOutput")

    with tile.TileContext(nc) as tc:
        _lowfreq_kernel(
            tc, out.ap(), delta.ap(), chtp.ap(), chtp3.ap(), cwt.ap(),
            ident.ap(), sumw.ap(), zeros.ap()
        )
    _USE_STOCK_TAIL = False
    if for_sim:
        return nc
    _strip_main_barrier(nc)
    _split_multi_waits(nc)
    _CACHED_NC = nc
    return nc


def _run(delta, **spmd_kwargs):
    import os
    os.environ["JAX_PLATFORMS"] = "axon"   # harness may have pinned cpu for the reference
    nc = _build()
    delta = np.ascontiguousarray(np.asarray(delta, dtype=np.float32))
    assert delta.shape == (B, C, H, W)
    shards = delta.reshape(N_CORES, IMGS_PER_CORE, H, W)
    in_maps = [
        {
            "delta": shards[i],
            "chtp": CHTP,
            "chtp3": CHTP3,
            "cwt": CWT,
            "ident": IDENT,
            "sumw": SUMW,
            "zeros": ZEROS,
        }
        for i in range(N_CORES)
    ]
    try:
        res = bass_utils.run_bass_kernel_spmd(
            nc, in_maps, core_ids=list(range(N_CORES)), **spmd_kwargs
        )
    except Exception:
        # transient NRT_EXEC_UNIT_UNRECOVERABLE has been observed on this
        # terminal; one retry typically succeeds.
        res = bass_utils.run_bass_kernel_spmd(
            nc, in_maps, core_ids=list(range(N_CORES)), **spmd_kwargs
        )
    total = np.float64(0.0)
    for r in res.results:
        total += np.float64(r["out"][0, 0])
    return np.float32(total).reshape(()), res


def kernel(delta):
    out, _ = _run(delta)
    return out



# revision 22
# speedup vs baseline: 1.7450x; 1.7450x over previous
"""Trainium2 Bass kernel for nn_LowFreqPenaltyLoss.

Computes mean(|einsum('ih,nchw,jw->ncij', Ch, delta, Cw)|) for
delta [256, 3, 256, 256] f32, Ch/Cw the 8x256 unnormalized DCT-II bases.

Strategy (data-parallel over batch, 8 cores):
  - each core gets 32 batches = 96 images [256, 256] (24 MiB), streamed via
    SWDGE DMAs that cast f32->bf16 inline. Partition p holds image rows 2p
    and 2p+1, so every DMA descriptor reads 2 KiB contiguous from HBM
    (~406 GB/s/core measured; the problem is memory-bound).
  - first 64 images go as four ~2 MiB group DMAs (one tiny 4-image lead DMA
    first so the first bytes hit the wire ~1us earlier); the last 32 go as
    four 1 MiB tiles so PE work keeps arriving inside the ~3.4us HAM
    activity window near the stream end (tail compute runs at 2.4 GHz, not
    1.2) and the post-stream critical path is a single 8-image tile.
  - stage A (contract h): psum[32q+i, (e,w)] += sum_hh ChT_hh[p,i].T @ img
    rows, 4 pairs packed per PSUM bank at partition offsets {0,32,64,96}
    via col tile_position; weights padded with zero columns to M=32.
  - copy bank -> SBUF (ACT, casts to bf16), PE-transpose 128x128 chunks,
    DVE copies out, stage B (contract w): out2[(q,i), j] += T.T @ CwT, then
    fused |.|+sum on DVE into a per-partition accumulator.
  - final: ones-matmul partition reduction scaled by 1/49152; host sums
    the 8 per-core partials. bf16 inputs + f32 PSUM accumulation give
    ~2e-4 relative error on the final scalar.

Known variance: SDMA engine 15 intermittently runs ~10% slower per packet
(observed on both DGE paths in ~half of runs), adding up to ~10us of
stream-tail drain. The engine<->partition swizzle is transfer-relative, so
work cannot be steered off it; the 2 KiB-descriptor layout halves its
packet count and roughly halves the penalty vs the 1 KiB layout.
"""

import sys

for _p in ("/root/.axon_site/_ro/trn_rl_repo", "/opt/trn_rl_repo"):
    if _p not in sys.path:
        sys.path.append(_p)

import numpy as np
from contextlib import ExitStack

import concourse.bass as bass
import concourse.tile as tile
from concourse import mybir, bass_utils
from concourse._compat import with_exitstack
from concourse.vector_clock import ScopedClock

# ---------------------------------------------------------------------------
# Workarounds for this image.
# ---------------------------------------------------------------------------

# walrus on this image rejects >1 sync-wait on one CTRL instruction; split the
# Tile exit-drain's waits across follow-up nops. The stock Tile exit tail
# (barrier + per-sem clear + barrier) is redundant with the NEFF-level fini
# (walrus emits its own all-engine barrier + full semaphore sweep), so keep
# only the drain + DMA completion waits.
_ORIG_DAB = tile.TileContext._drain_and_barrier
_USE_STOCK_TAIL = False


def _patched_drain_and_barrier(self, tick_clock, wait_clock):
    if _USE_STOCK_TAIL:
        return _ORIG_DAB(self, tick_clock, wait_clock)
    nc = self.nc
    drain_inst = nc.sync.drain()
    wait_clock.add_sem_waits(
        drain_inst.ins, ScopedClock({None: tick_clock.global_clock})
    )
    si = drain_inst.ins.sync_info
    waits = list(si.on_wait) if si and si.on_wait else []
    if len(waits) > 1:
        drain_inst.ins.sync_info = mybir.SyncInfo(
            on_wait=[waits[0]], on_update=list(si.on_update or [])
        )
        # Spread the remaining waits round-robin across engines: each NOP
        # costs ~60ns of issue latency and a serial ladder on one engine
        # adds ~1us to the tail; the NEFF fini barrier joins all engines
        # anyway, so any engine can carry any wait.
        wait_engines = [nc.sync, nc.vector, nc.scalar, nc.tensor, nc.gpsimd]
        for k, w in enumerate(waits[1:]):
            eng = wait_engines[k % len(wait_engines)]
            nop = eng.nop(nofuse=True, hint="drain_wait_split")
            nop.ins.sync_info = mybir.SyncInfo(on_wait=[w], on_update=[])
    popped = nc._tile_sem_poison_stack.pop()
    assert popped is self._sem_poison


tile.TileContext._drain_and_barrier = _patched_drain_and_barrier

# zero-egress container: profiling artifact upload must stay local.
bass_utils.upload_artifacts = lambda d: d


def _strip_main_barrier(nc):
    """Drop the prologue/epilogue all-engine barriers in 'main' plus the dead
    const-ap memsets. The barriers' only role is to fence the framework
    preamble (per-engine table loads) from the kernel, but per-engine program
    order already covers the table loads; nothing reads the const tiles
    (verified: only the 4 Memsets reference 'const-*' in the BIR). The first
    main-block instruction defines the profiler's first_useful_time, so the
    memsets also pad the measured window. Saves ~3-4us total."""
    for fn in nc.m.functions:
        for bb in fn.blocks:
            if bb.name != "main":
                continue
            bb.instructions[:] = [
                i for i in bb.instructions
                if not isinstance(
                    i,
                    (mybir.InstEventSemaphore, mybir.InstDrain, mybir.InstMemset),
                )
            ]


def _split_multi_waits(nc):
    """walrus on this image rejects >1 sync-wait per instruction: hoist extra
    waits onto fresh NoOps inserted just before, on the same engine."""
    for fn in nc.m.functions:
        for bb in fn.blocks:
            new_insts = []
            for inst in bb.instructions:
                si = inst.sync_info
                waits = list(si.on_wait) if si and si.on_wait else []
                if len(waits) > 1:
                    for w in waits[:-1]:
                        nop = mybir.InstNoOp(
                            name=nc.get_next_instruction_name(),
                            sync_info=mybir.SyncInfo(on_wait=[w], on_update=[]),
                            bass_nofuse=True,
                            engine=inst.engine,
                        )
                        new_insts.append(nop)
                    inst.sync_info = mybir.SyncInfo(
                        on_wait=[waits[-1]], on_update=list(si.on_update or [])
                    )
                new_insts.append(inst)
            bb.instructions[:] = new_insts

# ---------------------------------------------------------------------------
# Problem constants (hardcoded; kernel.py must be self-contained).
# ---------------------------------------------------------------------------

B, C, H, W = 256, 3, 256, 256
LOW_A = LOW_B = 8
N_CORES = 8
IMGS_PER_CORE = (B // N_CORES) * C          # 96
TOTAL_LOW = B * C * LOW_A * LOW_B           # 49152 -> mean divisor

F32 = mybir.dt.float32
BF16 = mybir.dt.bfloat16


def _dct_basis(K, N):
    n = np.arange(N, dtype=np.float64)
    k = np.arange(K, dtype=np.float64)
    return (2.0 * np.cos(np.pi * (2.0 * n[None, :] + 1.0) * k[:, None] / (2.0 * N))).astype(
        np.float32
    )


def _make_consts():
    Ch = _dct_basis(LOW_A, H)   # [8, 256]
    Cw = _dct_basis(LOW_B, W)   # [8, 256]
    # Rows are loaded 2-per-partition (partition p holds image rows 2p and
    # 2p+1 -> 2 KiB contiguous HBM reads per descriptor instead of 1 KiB).
    # chtp[hh, p, i] = Ch[i, 2p+hh], padded to 32 cols with zeros; the h
    # contraction becomes two accumulating passes hh=0,1 over 128 partitions.
    chtp = np.zeros((2, 128, 32), np.float32)
    for hh in range(2):
        chtp[hh, :, :8] = Ch[:, hh::2].T
    # cwt[wc, p, j] = Cw[j, wc*128+p]
    cwt = np.zeros((2, 128, 8), np.float32)
    for wc in range(2):
        cwt[wc] = Cw[:, wc * 128:(wc + 1) * 128].T
    import ml_dtypes
    bf16 = ml_dtypes.bfloat16
    ident = np.eye(128, dtype=bf16)
    sumw = np.full((128, 1), 1.0 / TOTAL_LOW, np.float32)
    return chtp.astype(bf16), cwt.astype(bf16), ident, sumw


CHTP, CWT, IDENT, SUMW = _make_consts()


# ---------------------------------------------------------------------------
# Kernel body (per core; SPMD over 8 cores).
# ---------------------------------------------------------------------------

@with_exitstack
def _lowfreq_kernel(ctx: ExitStack, tc, out_ap, delta_ap, chtp_ap, cwt_ap,
                    ident_ap, sumw_ap):
    nc = tc.nc

    const_pool = ctx.enter_context(tc.tile_pool(name="const", bufs=1))
    in_pool = ctx.enter_context(tc.tile_pool(name="input", bufs=4))
    tail_pool = ctx.enter_context(tc.tile_pool(name="tail", bufs=4))
    sS_pool = ctx.enter_context(tc.tile_pool(name="sS", bufs=3))
    tS_pool = ctx.enter_context(tc.tile_pool(name="tS", bufs=3))
    red_pool = ctx.enter_context(tc.tile_pool(name="red", bufs=2))
    acc_pool = ctx.enter_context(tc.tile_pool(name="acc", bufs=1))
    psA_pool = ctx.enter_context(tc.tile_pool(name="psA", bufs=3, space="PSUM"))
    psT_pool = ctx.enter_context(tc.tile_pool(name="psT", bufs=3, space="PSUM"))
    ps2_pool = ctx.enter_context(tc.tile_pool(name="ps2", bufs=2, space="PSUM"))

    # constants — issued on the ACT HWDGE ring (nc.scalar) so they neither
    # occupy the Sync ring nor delay the gpsimd (SWDGE) input stream.
    chtp = const_pool.tile([128, 2, 32], BF16)      # [p, hh, i]
    nc.scalar.dma_start(chtp[:], chtp_ap.rearrange("hh p i -> p hh i"))
    cwt = const_pool.tile([128, 2, 8], BF16)        # [p, wc, j]
    nc.scalar.dma_start(cwt[:], cwt_ap.rearrange("wc p j -> p wc j"))
    ident = const_pool.tile([128, 128], BF16)
    nc.scalar.dma_start(ident[:], ident_ap)
    sumw = const_pool.tile([128, 1], F32)
    nc.scalar.dma_start(sumw[:], sumw_ap)

    acc = acc_pool.tile([128, 1], F32)
    nc.vector.memset(acc[:], 0.0)

    def stage_bank(pair_rhs):
        """Stage A for one bank: pair_rhs(qq, hh) -> rhs AP for that column
        group and pass. Returns the filled PSUM bank."""
        bankA = psA_pool.tile([128, 512], F32)
        for hh in range(2):
            for qq in range(4):
                nc.tensor.matmul(
                    bankA[32 * qq:32 * qq + 32, :],
                    lhsT=chtp[:, hh, :],
                    rhs=pair_rhs(qq, hh),
                    start=(hh == 0),
                    stop=(hh == 1),
                    tile_position=(0, 32 * qq),
                    # CoreSim's zero-region tracker is bank-granular and
                    # flags the four concurrent per-partition col-groups;
                    # HW has_written state is per-element (verified on HW).
                    skip_group_check=True,
                )
        return bankA

    def finish_bank(bankA):
        """PSUM bank -> |.|-summed per-partition accumulator contribution."""
        # PSUM -> SBUF with f32->bf16 cast (ACT engine)
        sS = sS_pool.tile([128, 512], BF16)
        nc.scalar.copy(sS[:], bankA[:])

        # stage B: 4 PE transposes (own PSUM tiles: transpose-mode output
        # must start at a bank boundary on HW), DVE copies out, then
        # contract w into ps2 (e0 -> cols 0:8, e1 -> cols 8:16)
        tps = []
        for c in range(4):
            tp = psT_pool.tile([128, 128], BF16, tag="tp")
            nc.tensor.transpose(
                tp[:],
                sS[:, 128 * c:128 * c + 128],
                ident[:],
            )
            tps.append(tp)
        tSb = tS_pool.tile([128, 512], BF16)
        for c in range(4):
            nc.vector.tensor_copy(tSb[:, 128 * c:128 * c + 128], tps[c][:])

        ps2 = ps2_pool.tile([128, 16], F32)
        for e in range(2):
            for wc in range(2):
                c = 2 * e + wc
                nc.tensor.matmul(
                    ps2[:, 8 * e:8 * e + 8],
                    lhsT=tSb[:, 128 * c:128 * c + 128],
                    rhs=cwt[:, wc, :],
                    start=(wc == 0),
                    stop=(wc == 1),
                )
        red = red_pool.tile([128, 1], F32)
        nc.vector.tensor_reduce(
            red[:], ps2[:], axis=mybir.AxisListType.X,
            op=mybir.AluOpType.add, apply_absolute_value=True,
        )
        nc.vector.tensor_add(acc[:], acc[:], red[:])

    def do_half(gsel, moff):
        finish_bank(stage_bank(lambda qq, hh: gsel[:, moff + qq, :, hh, :]))

    # Loads: [p, m(pair), e, hh, w]. Partition p holds image rows 2p and
    # 2p+1 so each DMA descriptor reads 2 KiB contiguous from HBM: half the
    # packets and per-packet overhead of the 1 KiB h-chunk layout, and half
    # the intermittent slow-SDMA-15 straggler lag. SWDGE casts f32->bf16
    # inline; HBM traffic is the f32 source either way.
    #
    # Group 0 is split 4+12 images: Q7 descriptor emission is ~0.8us/MiB,
    # so a small lead DMA puts first bytes on the wire ~1us earlier. The
    # last 32 images go as four 1 MiB tiles (see module docstring).
    lead = tail_pool.tile([128, 2, 2, 2, 256], BF16, tag="lead")
    nc.gpsimd.dma_start(
        lead[:],
        delta_ap[0:4, :, :].rearrange("(m e) (p hh) w -> p (m e) (hh w)",
                                      m=2, e=2, p=128, hh=2),
    )
    gt0 = in_pool.tile([128, 6, 2, 2, 256], BF16, tag="g0")
    nc.gpsimd.dma_start(
        gt0[:],
        delta_ap[4:16, :, :].rearrange("(m e) (p hh) w -> p (m e) (hh w)",
                                       m=6, e=2, p=128, hh=2),
    )
    # group 0 first half: pairs {lead 0, lead 1, gt0 0, gt0 1}
    finish_bank(stage_bank(
        lambda qq, hh: (lead[:, qq, :, hh, :] if qq < 2
                        else gt0[:, qq - 2, :, hh, :])
    ))
    do_half(gt0, 2)

    for g in range(1, 4):
        gt = in_pool.tile([128, 8, 2, 2, 256], BF16)
        src = delta_ap[16 * g:16 * g + 16, :, :]
        nc.gpsimd.dma_start(
            gt[:],
            src.rearrange("(m e) (p hh) w -> p (m e) (hh w)",
                          m=8, e=2, p=128, hh=2),
        )
        for half in range(2):
            do_half(gt, 4 * half)
    for t in range(4):
        gth = tail_pool.tile([128, 4, 2, 2, 256], BF16)
        src = delta_ap[64 + 8 * t:64 + 8 * t + 8, :, :]
        nc.gpsimd.dma_start(
            gth[:],
            src.rearrange("(m e) (p hh) w -> p (m e) (hh w)",
                          m=4, e=2, p=128, hh=2),
        )
        do_half(gth, 0)

    # final partition reduction: out = acc.T @ sumw = sum_p acc[p] / 49152
    fout = ps2_pool.tile([1, 1], F32, tag="ps2")
    nc.tensor.matmul(fout[:], lhsT=acc[:], rhs=sumw[:], start=True, stop=True)
    fsb = red_pool.tile([1, 1], F32)
    nc.vector.tensor_copy(fsb[:], fout[:])
    nc.sync.dma_start(out_ap, fsb[:])


# ---------------------------------------------------------------------------
# Build + run.
# ---------------------------------------------------------------------------

_CACHED_NC = None


def _build(for_sim=False):
    global _CACHED_NC, _USE_STOCK_TAIL
    if not for_sim and _CACHED_NC is not None:
        return _CACHED_NC
    _USE_STOCK_TAIL = for_sim
    nc = bass.Bass("TRN2", target_bir_lowering=False, debug=False)
    delta = nc.dram_tensor("delta", [IMGS_PER_CORE, H, W], F32, kind="ExternalInput")
    chtp = nc.dram_tensor("chtp", list(CHTP.shape), BF16, kind="ExternalInput")
    cwt = nc.dram_tensor("cwt", list(CWT.shape), BF16, kind="ExternalInput")
    ident = nc.dram_tensor("ident", list(IDENT.shape), BF16, kind="ExternalInput")
    sumw = nc.dram_tensor("sumw", list(SUMW.shape), F32, kind="ExternalInput")
    out = nc.dram_tensor("out", [1, 1], F32, kind="ExternalOutput")

    with tile.TileContext(nc) as tc:
        _lowfreq_kernel(
            tc, out.ap(), delta.ap(), chtp.ap(), cwt.ap(), ident.ap(), sumw.ap()
        )
    _USE_STOCK_TAIL = False
    if for_sim:
        return nc
    _strip_main_barrier(nc)
    _split_multi_waits(nc)
    _CACHED_NC = nc
    return nc


def _run(delta, **spmd_kwargs):
    import os
    os.environ["JAX_PLATFORMS"] = "axon"   # harness may have pinned cpu for the reference
    nc = _build()
    delta = np.ascontiguousarray(np.asarray(delta, dtype=np.float32))
    assert delta.shape == (B, C, H, W)
    shards = delta.reshape(N_CORES, IMGS_PER_CORE, H, W)
    in_maps = [
        {
            "delta": shards[i],
            "chtp": CHTP,
            "cwt": CWT,
            "ident": IDENT,
            "sumw": SUMW,
        }
        for i in range(N_CORES)
    ]
    try:
        res = bass_utils.run_bass_kernel_spmd(
            nc, in_maps, core_ids=list(range(N_CORES)), **spmd_kwargs
        )
    except Exception:
        # transient NRT_EXEC_UNIT_UNRECOVERABLE has been observed on this
        # terminal; one retry typically succeeds.
        res = bass_utils.run_bass_kernel_spmd(
            nc, in_maps, core_ids=list(range(N_CORES)), **spmd_kwargs
        )
    total = np.float64(0.0)
    for r in res.results:
        total += np.float64(r["out"][0, 0])
    return np.float32(total).reshape(()), res


def kernel(delta):
    out, _ = _run(delta)
    return out
